# revision 14
# baseline (speedup 1.0000x reference)
"""Custom GRU cell kernel for Trainium2, data-parallel over batch on 8 NeuronCores.

v3: latency-optimized recurrence. Total time ~= T * L where L is the serial
per-step chain, so the design minimizes L:

  h_t = m2_t - m1_t with m1_t = (uhat_t - 1) * h_{t-1}, m2_t = uhat_t * htil_t.
  By linearity  U h_t = U m2_t + (-U) m1_t, and m1 is ready *before* tanh, so
  only the three U_* m2 matmuls sit on the chain; the (-U_*) m1 matmuls, the
  x-side matmuls, sigma_u, uhat, m1, and h_t itself all run off-chain.

  Chain per step:  m2 (DVE) -> U_r m2 (PE) -> sigma_r (ACT) -> t1 = r*mmh (DVE)
                   -> ident-MM (PE) -> tanh (ACT) -> m2 ...

Two independent batch substreams of 128 columns run concurrently (engines have
~50% slack per chain), so the full 256-column batch advances one step per L.

PSUM layout per substream per step (all f32), one reader set per bank so the
bank-overlap tracker never serializes independent reads:
  Four single-reader banks per substream (8 total, all bufs=1; each bank has
  exactly one accumulation group per step - start on its first matmul, stop
  on its last - and is read once after the stop):
  pr [128,128] r (sigma_r), pz [128,128] z (sigma_u),
  pmm [128,128] mmh (t1), pxh [128,128] xh (tanh)
  r:  W_r x (start) + (-U_r) m1 + U_r m2 (stop)     [t=0: U_r h0]
  z:  same with W_z/U_z
  mmh: (-U_h) m1 (start) + U_h m2 (stop)            [t=0: U_h h0]
  xh: W_h x (start) + I t1 (stop)
`a` is host-broadcast to [128, T, BL] so uhat = u * a is a bf16 SBUF op;
m1 and h_t run on GPSIMD (both off-chain; h_t feeds only m1/output, and m1
feeds the next step's PE matmuls + h computation, all GPSIMD-local).
"""

import sys

sys.path.insert(0, "/opt/trn_rl_repo")

import numpy as np
import ml_dtypes

import concourse.bass as bass  # noqa: F401  (import registers rust bindings)
import concourse.mybir as mybir
import concourse.tile as tile
from concourse import bacc
from concourse.bass_utils import run_bass_kernel_spmd

BF16 = mybir.dt.bfloat16
F32 = mybir.dt.float32
AF = mybir.ActivationFunctionType
OP = mybir.AluOpType

B, T, U = 2048, 200, 128
NCORES = 8
BL = B // NCORES  # 256 batch rows per core
NS = 2  # substreams per core
SW = BL // NS  # 128 batch columns per substream
TC = 25  # timesteps per chunk
NCHUNK = T // TC

M1_ENGINE = "dve"  # "dve" (STT unsupported on Pool engine)
FUSE_SIGMA = False  # fuse sigma over [r|z] (shorter ACT busy, longer chain)

PROFILE = False
LAST_RESULT = None
LAST_IN_MAPS = None

_cache = {}


def _build(has_brz: bool, T_=T, TC_=TC, BL_=BL, reps=1):
    NCHUNK_ = T_ // TC_
    nc = bacc.Bacc("TRN2", target_bir_lowering=False)

    xt = nc.dram_tensor("xt", [U, T_, BL_], BF16, kind="ExternalInput")
    arep = nc.dram_tensor("arep", [U, T_, BL_], BF16, kind="ExternalInput")
    h0t = nc.dram_tensor("h0t", [U, BL_], BF16, kind="ExternalInput")
    # wcat: W_r, U_r, W_z, U_z, W_h, U_h, -U_r, -U_z, -U_h
    wcat = nc.dram_tensor("wcat", [9, U, U], BF16, kind="ExternalInput")
    ident_d = nc.dram_tensor("ident", [U, U], BF16, kind="ExternalInput")
    biases = nc.dram_tensor("biases", [U, 3], F32, kind="ExternalInput")
    outt = nc.dram_tensor("outt", [U, T_, BL_], BF16, kind="ExternalOutput")

    with tile.TileContext(nc) as tc:
        with (
            tc.tile_pool(name="const", bufs=1) as cpool,
            tc.tile_pool(name="xchunk", bufs=2) as xpool,
            tc.tile_pool(name="achunk", bufs=2) as apool,
            tc.tile_pool(name="ochunk", bufs=2) as opool,
            tc.tile_pool(name="work", bufs=3) as wpool,
            tc.tile_pool(name="ppr", bufs=1, space="PSUM") as prpool,
            tc.tile_pool(name="pmm", bufs=1, space="PSUM") as pmmpool,
            tc.tile_pool(name="ppz", bufs=1, space="PSUM") as pzpool,
            tc.tile_pool(name="pxh", bufs=1, space="PSUM") as pxhpool,
        ):
            wts = []
            for i in range(9):
                wt = cpool.tile([U, U], BF16, tag=f"w{i}")
                nc.sync.dma_start(wt[:], wcat[i])
                wts.append(wt)
            w_r, u_r, w_z, u_z, w_h, u_h, un_r, un_z, un_h = wts
            ident = cpool.tile([U, U], BF16, tag="ident")
            nc.sync.dma_start(ident[:], ident_d[:])
            btile = cpool.tile([U, 3], F32, tag="biases")
            nc.sync.dma_start(btile[:], biases[:])
            b_r_ap = btile[:, 0:1]
            b_z_ap = btile[:, 1:2]
            b_h_ap = btile[:, 2:3]
            h0tile = cpool.tile([U, BL_], BF16, tag="h0")
            nc.sync.dma_start(h0tile[:], h0t[:])

            for _rep in range(reps):
                xchs = {}
                ochs = {}
                pending = [None] * NS  # (ps1, ps2) for the next finalize
                half = [None] * NS
                h_prev = [h0tile[:, s * SW : (s + 1) * SW] for s in range(NS)]
                m2_prev = [None] * NS
                m1_prev = [None] * NS

                def load_chunk(k):
                    if k >= NCHUNK_ or k in xchs:
                        return
                    t0, t1x = k * TC_, (k + 1) * TC_
                    xch = xpool.tile([U, TC_, BL_], BF16, tag="xch", name=f"xch{k}")
                    nc.sync.dma_start(xch[:], xt[:, t0:t1x, :])
                    ach = apool.tile([U, TC_, BL_], BF16, tag="ach", name=f"ach{k}")
                    nc.sync.dma_start(ach[:], arep[:, t0:t1x, :])
                    xchs[k] = (xch, ach)

                def get_och(k):
                    if k not in ochs:
                        ochs[k] = opool.tile(
                            [U, TC_, BL_], BF16, tag="och", name=f"och{k}"
                        )
                    return ochs[k]

                def emit_xside(s, t):
                    """x-side matmuls + early (-U_*) m1 matmuls for step t."""
                    if t >= T_:
                        return
                    k, dt = divmod(t, TC_)
                    xch, _ach = xchs[k]
                    xs = xch[:, dt, s * SW : (s + 1) * SW]
                    pr = prpool.tile([U, SW], F32, tag=f"pr_{s}", name=f"pr_{s}_{t}")
                    nc.tensor.matmul(pr[:], w_r[:], xs, start=True, stop=False)
                    pz = pzpool.tile([U, SW], F32, tag=f"pz_{s}", name=f"pz_{s}_{t}")
                    nc.tensor.matmul(pz[:], w_z[:], xs, start=True, stop=False)
                    pxh = pxhpool.tile([U, SW], F32, tag=f"pxh_{s}", name=f"pxh_{s}_{t}")
                    nc.tensor.matmul(pxh[:], w_h[:], xs, start=True, stop=False)
                    pending[s] = (pr, pz, pxh)

                def emit_h1(s, t):
                    """Finalize matmuls + gates + t1/uhat/m1 for step t."""
                    k, dt = divmod(t, TC_)
                    if dt == 0:
                        load_chunk(k + 1)
                        get_och(k)
                    scol = slice(s * SW, (s + 1) * SW)
                    pr, pz, pxh = pending[s]
                    pmm = pmmpool.tile([U, SW], F32, tag=f"pmm_{s}", name=f"pmm_{s}_{t}")
                    _xch, ach = xchs[k]

                    # h_{t-1} = m2 - m1 (both ready here): r-pair first (its
                    # bank gates the chain's sigma_r), then mm-pair (gates the
                    # off-chain copy), then z-pair.
                    if t == 0:
                        rhs = h_prev[s]
                        nc.tensor.matmul(pr[:], u_r[:], rhs, start=False, stop=True)
                        nc.tensor.matmul(pmm[:], u_h[:], rhs, start=True, stop=True)
                        nc.tensor.matmul(pz[:], u_z[:], rhs, start=False, stop=True)
                    else:
                        m2p, m1p = m2_prev[s], m1_prev[s]
                        nc.tensor.matmul(pr[:], u_r[:], m2p, start=False, stop=False)
                        nc.tensor.matmul(pr[:], un_r[:], m1p, start=False, stop=True)
                        nc.tensor.matmul(pmm[:], u_h[:], m2p, start=True, stop=False)
                        nc.tensor.matmul(pmm[:], un_h[:], m1p, start=False, stop=True)
                        nc.tensor.matmul(pz[:], u_z[:], m2p, start=False, stop=False)
                        nc.tensor.matmul(pz[:], un_z[:], m1p, start=False, stop=True)

                    r_sb = wpool.tile([U, SW], BF16, tag=f"r{s}", name=f"r{s}_{t}")
                    u_sb = wpool.tile([U, SW], BF16, tag=f"u{s}", name=f"u{s}_{t}")
                    if has_brz:
                        nc.scalar.activation(r_sb[:], pr[:], AF.Sigmoid, bias=b_r_ap)
                        nc.scalar.activation(u_sb[:], pz[:], AF.Sigmoid, bias=b_z_ap)
                    else:
                        nc.scalar.activation(r_sb[:], pr[:], AF.Sigmoid)
                        nc.scalar.activation(u_sb[:], pz[:], AF.Sigmoid)

                    t1 = wpool.tile([U, SW], BF16, tag=f"t1_{s}", name=f"t1_{s}_{t}")
                    nc.vector.tensor_tensor(t1[:], pmm[:], r_sb[:], OP.mult)
                    uhat = wpool.tile([U, SW], BF16, tag=f"uhat{s}", name=f"uhat{s}_{t}")
                    nc.vector.tensor_tensor(uhat[:], u_sb[:], ach[:, dt, scol], OP.mult)
                    m1 = wpool.tile([U, SW], BF16, tag=f"m1_{s}", name=f"m1_{s}_{t}")
                    nc.vector.scalar_tensor_tensor(
                        m1[:], uhat[:], 1.0, h_prev[s], OP.subtract, OP.mult
                    )
                    half[s] = (t, pxh, t1, uhat, m1)

                def emit_h2(s):
                    """ident-MM, tanh, m2, h_t, next x-side for substream s."""
                    t, pxh, t1, uhat, m1 = half[s]
                    k, dt = divmod(t, TC_)
                    scol = slice(s * SW, (s + 1) * SW)
                    och = get_och(k)

                    nc.tensor.matmul(pxh[:], ident[:], t1[:], start=False, stop=True)

                    htil = wpool.tile([U, SW], BF16, tag=f"htil{s}", name=f"htil{s}_{t}")
                    if has_brz:
                        nc.scalar.activation(htil[:], pxh[:], AF.Tanh, bias=b_h_ap)
                    else:
                        nc.scalar.activation(htil[:], pxh[:], AF.Tanh)

                    m2 = wpool.tile([U, SW], BF16, tag=f"m2_{s}", name=f"m2_{s}_{t}")
                    nc.vector.tensor_tensor(m2[:], uhat[:], htil[:], OP.mult)
                    hn = och[:, dt, scol]
                    # h_t is off-chain (only the output + next m1 need it);
                    # gpsimd keeps it out of the DVE queue
                    nc.gpsimd.tensor_tensor(hn, m2[:], m1[:], OP.subtract)

                    m2_prev[s] = m2[:]
                    m1_prev[s] = m1[:]
                    h_prev[s] = hn
                    # next step's x-side + early m1-matmuls (ps1/ps2 of t+1)
                    emit_xside(s, t + 1)

                    if s == NS - 1 and dt == TC_ - 1:
                        nc.sync.dma_start(outt[:, k * TC_ : (k + 1) * TC_, :], och[:])
                        xchs.pop(k, None)

                load_chunk(0)
                for s in range(NS):
                    emit_xside(s, 0)
                emit_h1(0, 0)
                for t in range(T_):
                    emit_h1(1, t)
                    emit_h2(0)
                    if t + 1 < T_:
                        emit_h1(0, t + 1)
                    emit_h2(1)

    nc.compile()
    return nc


def kernel(inputs, h0, W_r, U_r, b_r, W_z, U_z, b_z, W_h, U_h, b_h):
    global LAST_RESULT, LAST_IN_MAPS
    inputs = np.asarray(inputs, dtype=np.float32)
    h0 = np.asarray(h0, dtype=np.float32)
    ws = [np.asarray(w, dtype=np.float32) for w in (W_r, U_r, W_z, U_z, W_h, U_h)]
    bs = [np.asarray(b, dtype=np.float32) for b in (b_r, b_z, b_h)]

    has_brz = bool(np.any(bs[0]) or np.any(bs[1]))
    key = has_brz
    if key not in _cache:
        _cache[key] = _build(has_brz)
    nc = _cache[key]

    bf = ml_dtypes.bfloat16
    wcat = np.stack(
        [w.astype(bf) for w in ws]
        + [(-ws[1]).astype(bf), (-ws[3]).astype(bf), (-ws[5]).astype(bf)]
    )  # [9, U, U]: W_r U_r W_z U_z W_h U_h -U_r -U_z -U_h
    # reorder to W_r, U_r, W_z, U_z, W_h, U_h, -U_r, -U_z, -U_h (build order)
    ident = np.eye(U, dtype=bf)
    biases = np.stack([bs[0], bs[1], bs[2]], axis=1).astype(np.float32)  # [U, 3]

    x = inputs[:, :, :U]  # [B, T, U]
    a = inputs[:, :, U]  # [B, T]

    in_maps = []
    for c in range(NCORES):
        sl = slice(c * BL, (c + 1) * BL)
        xt_c = np.ascontiguousarray(x[sl].transpose(2, 1, 0)).astype(bf)  # [U,T,BL]
        a_tb = a[sl].T.astype(bf)  # [T, BL]
        arep_c = np.ascontiguousarray(
            np.broadcast_to(a_tb[None, :, :], (U, T, BL))
        )  # [U,T,BL]
        h0t_c = np.ascontiguousarray(h0[sl].T).astype(bf)  # [U, BL]
        in_maps.append(
            {
                "xt": xt_c,
                "arep": arep_c,
                "h0t": h0t_c,
                "wcat": wcat,
                "ident": ident,
                "biases": biases,
            }
        )

    res = run_bass_kernel_spmd(nc, in_maps, list(range(NCORES)), trace=PROFILE)
    LAST_IN_MAPS = in_maps
    LAST_RESULT = res

    out = np.empty((B, T, U), dtype=np.float32)
    for c in range(NCORES):
        sl = slice(c * BL, (c + 1) * BL)
        out[sl] = res.results[c]["outt"].astype(np.float32).transpose(2, 1, 0)
    return out


# revision 15
# speedup vs baseline: 1.1738x; 1.1738x over previous
"""Custom GRU cell kernel for Trainium2, data-parallel over batch on 8 NeuronCores.

v3: latency-optimized recurrence. Total time ~= T * L where L is the serial
per-step chain, so the design minimizes L:

  h_t = m2_t - m1_t with m1_t = (uhat_t - 1) * h_{t-1}, m2_t = uhat_t * htil_t.
  By linearity  U h_t = U m2_t + (-U) m1_t, and m1 is ready *before* tanh, so
  only the three U_* m2 matmuls sit on the chain; the (-U_*) m1 matmuls, the
  x-side matmuls, sigma_u, uhat, m1, and h_t itself all run off-chain.

  Chain per step:  m2 (DVE) -> U_r m2 (PE) -> sigma_r (ACT) -> t1 = r*mmh (DVE)
                   -> ident-MM (PE) -> tanh (ACT) -> m2 ...

Two independent batch substreams of 128 columns run concurrently (engines have
~50% slack per chain), so the full 256-column batch advances one step per L.

PSUM layout per substream per step (all f32), one reader set per bank so the
bank-overlap tracker never serializes independent reads:
  Four single-reader banks per substream (8 total, all bufs=1; each bank has
  exactly one accumulation group per step - start on its first matmul, stop
  on its last - and is read once after the stop):
  pr [128,128] r (sigma_r), pz [128,128] z (sigma_u),
  pmm [128,128] mmh (t1), pxh [128,128] xh (tanh)
  r:  W_r x (start) + (-U_r) m1 + U_r m2 (stop)     [t=0: U_r h0]
  z:  same with W_z/U_z
  mmh: (-U_h) m1 (start) + U_h m2 (stop)            [t=0: U_h h0]
  xh: W_h x (start) + I t1 (stop)
`a` is host-broadcast to [128, T, BL] so uhat = u * a is a bf16 SBUF op;
m1 and h_t run on GPSIMD (both off-chain; h_t feeds only m1/output, and m1
feeds the next step's PE matmuls + h computation, all GPSIMD-local).
"""

import sys

sys.path.insert(0, "/opt/trn_rl_repo")

import numpy as np
import ml_dtypes

import concourse.bass as bass  # noqa: F401  (import registers rust bindings)
import concourse.mybir as mybir
import concourse.tile as tile
from concourse import bacc
from concourse.bass_utils import run_bass_kernel_spmd

BF16 = mybir.dt.bfloat16
F32 = mybir.dt.float32
AF = mybir.ActivationFunctionType
OP = mybir.AluOpType

B, T, U = 2048, 200, 128
NCORES = 8
BL = B // NCORES  # 256 batch rows per core
NS = 2  # substreams per core
SW = BL // NS  # 128 batch columns per substream
TC = 25  # timesteps per chunk
NCHUNK = T // TC

M1_ENGINE = "dve"  # "dve" (STT unsupported on Pool engine)
FUSE_SIGMA = False  # fuse sigma over [r|z] (shorter ACT busy, longer chain)

PROFILE = False
LAST_RESULT = None
LAST_IN_MAPS = None

_cache = {}


def _build(has_brz: bool, T_=T, TC_=TC, BL_=BL, reps=1):
    NCHUNK_ = T_ // TC_
    nc = bacc.Bacc("TRN2", target_bir_lowering=False)

    xt = nc.dram_tensor("xt", [U, T_, BL_], BF16, kind="ExternalInput")
    arep = nc.dram_tensor("arep", [U, T_, BL_], BF16, kind="ExternalInput")
    h0t = nc.dram_tensor("h0t", [U, BL_], BF16, kind="ExternalInput")
    # wcat: W_r, U_r, W_z, U_z, W_h, U_h, -U_r, -U_z, -U_h
    wcat = nc.dram_tensor("wcat", [9, U, U], BF16, kind="ExternalInput")
    ident_d = nc.dram_tensor("ident", [U, U], BF16, kind="ExternalInput")
    biases = nc.dram_tensor("biases", [U, 3], F32, kind="ExternalInput")
    outt = nc.dram_tensor("outt", [U, T_, BL_], BF16, kind="ExternalOutput")

    with tile.TileContext(nc) as tc:
        with (
            tc.tile_pool(name="const", bufs=1) as cpool,
            tc.tile_pool(name="xchunk", bufs=2) as xpool,
            tc.tile_pool(name="achunk", bufs=2) as apool,
            tc.tile_pool(name="ochunk", bufs=2) as opool,
            tc.tile_pool(name="work", bufs=3) as wpool,
            tc.tile_pool(name="ppr", bufs=1, space="PSUM") as prpool,
            tc.tile_pool(name="pmm", bufs=1, space="PSUM") as pmmpool,
            tc.tile_pool(name="ppz", bufs=1, space="PSUM") as pzpool,
            tc.tile_pool(name="pxh", bufs=1, space="PSUM") as pxhpool,
        ):
            wts = []
            for i in range(9):
                wt = cpool.tile([U, U], BF16, tag=f"w{i}")
                nc.sync.dma_start(wt[:], wcat[i])
                wts.append(wt)
            w_r, u_r, w_z, u_z, w_h, u_h, un_r, un_z, un_h = wts
            ident = cpool.tile([U, U], BF16, tag="ident")
            nc.sync.dma_start(ident[:], ident_d[:])
            btile = cpool.tile([U, 3], F32, tag="biases")
            nc.sync.dma_start(btile[:], biases[:])
            b_r_ap = btile[:, 0:1]
            b_z_ap = btile[:, 1:2]
            b_h_ap = btile[:, 2:3]
            h0tile = cpool.tile([U, BL_], BF16, tag="h0")
            nc.sync.dma_start(h0tile[:], h0t[:])

            for _rep in range(reps):
                xchs = {}
                ochs = {}
                pending = [None] * NS  # (ps1, ps2) for the next finalize
                half = [None] * NS
                h_prev = [h0tile[:, s * SW : (s + 1) * SW] for s in range(NS)]
                m2_prev = [None] * NS
                m1_prev = [None] * NS

                def load_chunk(k):
                    if k >= NCHUNK_ or k in xchs:
                        return
                    t0, t1x = k * TC_, (k + 1) * TC_
                    xch = xpool.tile([U, TC_, BL_], BF16, tag="xch", name=f"xch{k}")
                    nc.sync.dma_start(xch[:], xt[:, t0:t1x, :])
                    ach = apool.tile([U, TC_, BL_], BF16, tag="ach", name=f"ach{k}")
                    nc.sync.dma_start(ach[:], arep[:, t0:t1x, :])
                    xchs[k] = (xch, ach)

                def get_och(k):
                    if k not in ochs:
                        ochs[k] = opool.tile(
                            [U, TC_, BL_], BF16, tag="och", name=f"och{k}"
                        )
                    return ochs[k]

                def emit_xside(s, t):
                    """x-side matmuls + early (-U_*) m1 matmuls for step t."""
                    if t >= T_:
                        return
                    k, dt = divmod(t, TC_)
                    xch, _ach = xchs[k]
                    xs = xch[:, dt, s * SW : (s + 1) * SW]
                    pr = prpool.tile([U, SW], F32, tag=f"pr_{s}", name=f"pr_{s}_{t}")
                    nc.tensor.matmul(pr[:], w_r[:], xs, start=True, stop=False)
                    pz = pzpool.tile([U, SW], F32, tag=f"pz_{s}", name=f"pz_{s}_{t}")
                    nc.tensor.matmul(pz[:], w_z[:], xs, start=True, stop=False)
                    pxh = pxhpool.tile([U, SW], F32, tag=f"pxh_{s}", name=f"pxh_{s}_{t}")
                    nc.tensor.matmul(pxh[:], w_h[:], xs, start=True, stop=False)
                    pending[s] = (pr, pz, pxh)

                def emit_h1(s, t):
                    """Finalize matmuls + gates + t1/uhat/m1 for step t."""
                    k, dt = divmod(t, TC_)
                    if dt == 0:
                        load_chunk(k + 1)
                        get_och(k)
                    scol = slice(s * SW, (s + 1) * SW)
                    pr, pz, pxh = pending[s]
                    pmm = pmmpool.tile([U, SW], F32, tag=f"pmm_{s}", name=f"pmm_{s}_{t}")
                    _xch, ach = xchs[k]

                    # h_{t-1} = m2 - m1 (both ready here): r-pair first (its
                    # bank gates the chain's sigma_r), then mm-pair (gates the
                    # off-chain copy), then z-pair.
                    if t == 0:
                        rhs = h_prev[s]
                        nc.tensor.matmul(pr[:], u_r[:], rhs, start=False, stop=True)
                        nc.tensor.matmul(pmm[:], u_h[:], rhs, start=True, stop=True)
                        nc.tensor.matmul(pz[:], u_z[:], rhs, start=False, stop=True)
                    else:
                        m2p, m1p = m2_prev[s], m1_prev[s]
                        nc.tensor.matmul(pr[:], u_r[:], m2p, start=False, stop=False)
                        nc.tensor.matmul(pr[:], un_r[:], m1p, start=False, stop=True)
                        nc.tensor.matmul(pmm[:], u_h[:], m2p, start=True, stop=False)
                        nc.tensor.matmul(pmm[:], un_h[:], m1p, start=False, stop=True)
                        nc.tensor.matmul(pz[:], u_z[:], m2p, start=False, stop=False)
                        nc.tensor.matmul(pz[:], un_z[:], m1p, start=False, stop=True)

                    r_sb = wpool.tile([U, SW], BF16, tag=f"r{s}", name=f"r{s}_{t}")
                    u_sb = wpool.tile([U, SW], BF16, tag=f"u{s}", name=f"u{s}_{t}")
                    if has_brz:
                        nc.scalar.activation(r_sb[:], pr[:], AF.Sigmoid, bias=b_r_ap)
                        nc.scalar.activation(u_sb[:], pz[:], AF.Sigmoid, bias=b_z_ap)
                    else:
                        nc.scalar.activation(r_sb[:], pr[:], AF.Sigmoid)
                        nc.scalar.activation(u_sb[:], pz[:], AF.Sigmoid)

                    t1 = wpool.tile([U, SW], BF16, tag=f"t1_{s}", name=f"t1_{s}_{t}")
                    nc.vector.tensor_tensor(t1[:], pmm[:], r_sb[:], OP.mult)
                    uhat = wpool.tile([U, SW], BF16, tag=f"uhat{s}", name=f"uhat{s}_{t}")
                    nc.vector.tensor_tensor(uhat[:], u_sb[:], ach[:, dt, scol], OP.mult)
                    m1 = wpool.tile([U, SW], BF16, tag=f"m1_{s}", name=f"m1_{s}_{t}")
                    nc.vector.scalar_tensor_tensor(
                        m1[:], uhat[:], 1.0, h_prev[s], OP.subtract, OP.mult
                    )
                    half[s] = (t, pxh, t1, uhat, m1)

                def emit_h2(s):
                    """ident-MM, tanh, m2, h_t, next x-side for substream s."""
                    t, pxh, t1, uhat, m1 = half[s]
                    k, dt = divmod(t, TC_)
                    scol = slice(s * SW, (s + 1) * SW)
                    och = get_och(k)

                    nc.tensor.matmul(pxh[:], ident[:], t1[:], start=False, stop=True)

                    htil = wpool.tile([U, SW], BF16, tag=f"htil{s}", name=f"htil{s}_{t}")
                    if has_brz:
                        nc.scalar.activation(htil[:], pxh[:], AF.Tanh, bias=b_h_ap)
                    else:
                        nc.scalar.activation(htil[:], pxh[:], AF.Tanh)

                    m2 = wpool.tile([U, SW], BF16, tag=f"m2_{s}", name=f"m2_{s}_{t}")
                    nc.vector.tensor_tensor(m2[:], uhat[:], htil[:], OP.mult)
                    hn = och[:, dt, scol]
                    nc.vector.tensor_tensor(hn, m2[:], m1[:], OP.subtract)

                    m2_prev[s] = m2[:]
                    m1_prev[s] = m1[:]
                    h_prev[s] = hn
                    # next step's x-side + early m1-matmuls (ps1/ps2 of t+1)
                    emit_xside(s, t + 1)

                    if s == NS - 1 and dt == TC_ - 1:
                        nc.sync.dma_start(outt[:, k * TC_ : (k + 1) * TC_, :], och[:])
                        xchs.pop(k, None)

                load_chunk(0)
                for s in range(NS):
                    emit_xside(s, 0)
                emit_h1(0, 0)
                for t in range(T_):
                    emit_h1(1, t)
                    emit_h2(0)
                    if t + 1 < T_:
                        emit_h1(0, t + 1)
                    emit_h2(1)

    nc.compile()
    return nc


def kernel(inputs, h0, W_r, U_r, b_r, W_z, U_z, b_z, W_h, U_h, b_h):
    global LAST_RESULT, LAST_IN_MAPS
    inputs = np.asarray(inputs, dtype=np.float32)
    h0 = np.asarray(h0, dtype=np.float32)
    ws = [np.asarray(w, dtype=np.float32) for w in (W_r, U_r, W_z, U_z, W_h, U_h)]
    bs = [np.asarray(b, dtype=np.float32) for b in (b_r, b_z, b_h)]

    has_brz = bool(np.any(bs[0]) or np.any(bs[1]))
    key = has_brz
    if key not in _cache:
        _cache[key] = _build(has_brz)
    nc = _cache[key]

    bf = ml_dtypes.bfloat16
    wcat = np.stack(
        [w.astype(bf) for w in ws]
        + [(-ws[1]).astype(bf), (-ws[3]).astype(bf), (-ws[5]).astype(bf)]
    )  # [9, U, U]: W_r U_r W_z U_z W_h U_h -U_r -U_z -U_h
    # reorder to W_r, U_r, W_z, U_z, W_h, U_h, -U_r, -U_z, -U_h (build order)
    ident = np.eye(U, dtype=bf)
    biases = np.stack([bs[0], bs[1], bs[2]], axis=1).astype(np.float32)  # [U, 3]

    x = inputs[:, :, :U]  # [B, T, U]
    a = inputs[:, :, U]  # [B, T]

    in_maps = []
    for c in range(NCORES):
        sl = slice(c * BL, (c + 1) * BL)
        xt_c = np.ascontiguousarray(x[sl].transpose(2, 1, 0)).astype(bf)  # [U,T,BL]
        a_tb = a[sl].T.astype(bf)  # [T, BL]
        arep_c = np.ascontiguousarray(
            np.broadcast_to(a_tb[None, :, :], (U, T, BL))
        )  # [U,T,BL]
        h0t_c = np.ascontiguousarray(h0[sl].T).astype(bf)  # [U, BL]
        in_maps.append(
            {
                "xt": xt_c,
                "arep": arep_c,
                "h0t": h0t_c,
                "wcat": wcat,
                "ident": ident,
                "biases": biases,
            }
        )

    res = run_bass_kernel_spmd(nc, in_maps, list(range(NCORES)), trace=PROFILE)
    LAST_IN_MAPS = in_maps
    LAST_RESULT = res

    out = np.empty((B, T, U), dtype=np.float32)
    for c in range(NCORES):
        sl = slice(c * BL, (c + 1) * BL)
        out[sl] = res.results[c]["outt"].astype(np.float32).transpose(2, 1, 0)
    return out
